# revision 2
# baseline (speedup 1.0000x reference)
"""Masked-BCE valid-region loss on 8 Trainium2 NeuronCores — bf16.

Host casts the three big tensors to bf16 (the kernel is DMA-feed
limited at ~340 GB/s/core) and packs per image as
[IMG, 128, {p,n,x}, 2048].

Engine split (measured on HW):
  DVE : pt = min(p,n) [2x] ; mask = (pt>0.5) [4x] ; xm = mask*x [2x]
  PE  : count = ones^T @ mask ; per-image sxm = ones^T @ xm  (PSUM acc)
  ACT : e = exp(xm) per chunk ; sp = ln(e+1) per GROUP of chunks with
        fused accum -> ssp (merging LNs cuts per-op overhead and
        READ_ACCUMULATOR count on the bottleneck engine)
  DVE : PSUM -> SBUF copies of count/sxm (overlap the final ACT passes)

The pipeline is feed-limited: exec ~= stream_end + dma_sem_lag +
dve(last chunk) + act(last chunk) + output. Hence: ramp in with img0
halves ([p,n] before [x] so min/mask overlap the x transfer), stream
img1/img2 as single 12KB-descriptor DMAs (best measured bandwidth),
taper img3 as a half then two quarters so the post-stream serial tail
is minimal.

Host combine (y constant per image):
    bce = softplus(x) - x*y
    sum(bce*m) = sum_masked softplus(x) - y * sum(x*m)
    softplus(x*m) = softplus(x) where m==1, ln(2) where m==0
 => sum_masked softplus(x) = sum softplus(x*m) - (N - count)*ln(2)
"""

import sys

for _p in ("/opt/trn_rl_repo", "/root/.axon_site/_ro/trn_rl_repo"):
    if _p not in sys.path:
        sys.path.append(_p)

import ml_dtypes
import numpy as np

import concourse.bacc as bacc
import concourse.tile as tile
from concourse import mybir
from concourse.bass_utils import run_bass_kernel_spmd

B, H, W = 32, 512, 512
N_CORES = 8
IMGS_PER_CORE = B // N_CORES  # 4
P = 128
FD = (H * W) // P  # 2048
N_PER_IMG = H * W  # 262144
HF = FD // 2  # 1024
QC = FD // 4  # 512
MM = 512  # matmul moving width == PSUM row width

# Compute chunks: (img, col_start, cols).
CHUNKS = [
    (0, 0, HF),
    (0, HF, HF),
    (1, 0, FD),
    (2, 0, FD),
    (3, 0, HF),
    (3, HF, QC),
    (3, HF + QC, QC),
]
# LN groups (indices into CHUNKS): early group fills ACT idle during the
# ramp; the last group is small so the post-stream tail stays short.
LN_GROUPS = [[0, 1], [2], [3, 4], [5, 6]]
N_COLS = len(LN_GROUPS)

_nc_cache = None


def _patch_act_tables():
    """Steer every activation to `natural_log_exp_and_others` so bacc
    emits a single ACT_TABLE_LOAD (exp and ln share that set)."""
    import concourse.hw_specs as hw_specs

    if getattr(bacc, "_act_tables_patched", False):
        return
    orig = hw_specs.get_activation_tables

    def patched(module_arch):
        tables = orig(module_arch)
        keep = "natural_log_exp_and_others"
        if keep in tables:
            tables = {
                name: (funcs if name == keep else set())
                for name, funcs in tables.items()
            }
        return tables

    bacc.get_activation_tables = patched
    bacc._act_tables_patched = True


def _build_bass():
    _patch_act_tables()
    f32 = mybir.dt.float32
    bf16 = mybir.dt.bfloat16
    nc = bacc.Bacc()
    xpn_d = nc.dram_tensor(
        "xpn", [IMGS_PER_CORE, P, 3, FD], bf16, kind="ExternalInput"
    )
    ssp_d = nc.dram_tensor("ssp", [P, N_COLS], f32, kind="ExternalOutput")
    # red[0, 0:2048] = per-image sum(x*m) (512-wide regions); [2048:2560] = count.
    red_d = nc.dram_tensor("red", [1, 5 * MM], f32, kind="ExternalOutput")

    chunk_group = {}
    group_offset = {}
    for gi, grp in enumerate(LN_GROUPS):
        off = 0
        for ci in grp:
            chunk_group[ci] = gi
            group_offset[ci] = off
            off += CHUNKS[ci][2]

    with tile.TileContext(nc) as tc:
        with (
            tc.tile_pool(name="io", bufs=1) as io_pool,
            tc.tile_pool(name="xm", bufs=len(CHUNKS)) as xm_pool,
            tc.tile_pool(name="stats", bufs=1) as stats_pool,
            tc.tile_pool(name="psum", bufs=1, space="PSUM") as psum_pool,
        ):
            ssp_t = stats_pool.tile([P, N_COLS], f32)
            ones = stats_pool.tile([P, 1], bf16)
            nc.vector.memset(ones, 1.0)
            sxm_ps = psum_pool.tile([1, 4 * MM], f32)
            cnt_ps = psum_pool.tile([1, MM], f32)
            # One exp-output tile per LN group; exp ops fill slices so a
            # single LN (one accum, one READ_ACCUMULATOR) covers the group.
            et_g = [
                stats_pool.tile(
                    [P, sum(CHUNKS[ci][2] for ci in grp)],
                    bf16,
                    name=f"etg{gi}",
                    tag=f"etg{gi}",
                )
                for gi, grp in enumerate(LN_GROUPS)
            ]

            # DMAs: img0 as pn/x half pairs (ramp), img1/img2 single
            # 12KB-descriptor transfers (bandwidth), img3 as pn/x half then
            # quarter pairs (short tail). FIFO on one HWDGE ring.
            chunk_tiles = []
            dmas = []
            for ci, (i, c0, cols) in enumerate(CHUNKS):
                if cols == FD:
                    t = io_pool.tile([P, 3, FD], bf16, tag=f"img{ci}")
                    dmas.append((t, xpn_d[i]))
                    chunk_tiles.append((t[:, 0, :], t[:, 1, :], t[:, 2, :]))
                else:
                    sl = slice(c0, c0 + cols)
                    tpn = io_pool.tile([P, 2, cols], bf16, tag=f"pn{ci}")
                    tx = io_pool.tile([P, cols], bf16, tag=f"x{ci}")
                    dmas.append((tpn, xpn_d[i][:, 0:2, sl]))
                    dmas.append((tx, xpn_d[i][:, 2, sl]))
                    chunk_tiles.append((tpn[:, 0, :], tpn[:, 1, :], tx))
            for out_t, in_ap in dmas:
                nc.sync.dma_start(out=out_t, in_=in_ap)

            total = sum(cols // MM for _, _, cols in CHUNKS)
            img_mm_total = {i: 0 for i in range(IMGS_PER_CORE)}
            for i, _, cols in CHUNKS:
                img_mm_total[i] += cols // MM
            mm_done = 0
            img_mm_done = {i: 0 for i in range(IMGS_PER_CORE)}
            ln_done = set()
            for ci, (i, c0, cols) in enumerate(CHUNKS):
                pt, nt, tx = chunk_tiles[ci]
                # pt = min(p, n); bf16 tensor_tensor runs 2x.
                nc.vector.tensor_tensor(
                    out=pt, in0=pt, in1=nt, op=mybir.AluOpType.min
                )
                # mask = (pt > 0.5) in bf16 {0,1}; plain tensor_scalar is 4x.
                nc.vector.tensor_scalar(
                    out=nt,
                    in0=pt,
                    scalar1=0.5,
                    scalar2=None,
                    op0=mybir.AluOpType.is_gt,
                )
                # xm = mask * x; bf16 tensor_tensor 2x.
                xmt = xm_pool.tile([P, cols], bf16, tag="xmt")
                nc.vector.tensor_tensor(
                    out=xmt, in0=nt, in1=tx, op=mybir.AluOpType.mult
                )
                # TensorE reductions into PSUM: count over mask (one global
                # accumulation group), sum(x*m) per image (one group/image).
                for c in range(cols // MM):
                    nc.tensor.matmul(
                        cnt_ps,
                        ones,
                        nt[:, c * MM : (c + 1) * MM],
                        start=(mm_done == 0),
                        stop=(mm_done == total - 1),
                    )
                    nc.tensor.matmul(
                        sxm_ps[:, i * MM : (i + 1) * MM],
                        ones,
                        xmt[:, c * MM : (c + 1) * MM],
                        start=(img_mm_done[i] == 0),
                        stop=(img_mm_done[i] == img_mm_total[i] - 1),
                    )
                    mm_done += 1
                    img_mm_done[i] += 1
                # e = exp(xm) into this chunk's slice of its group tile.
                gi = chunk_group[ci]
                off = group_offset[ci]
                nc.scalar.activation(
                    out=et_g[gi][:, off : off + cols],
                    in_=xmt,
                    func=mybir.ActivationFunctionType.Exp,
                )
                # Once a group's last chunk is exp'd, one LN covers the
                # whole group: sp = ln(e+1), accum -> ssp column.
                if ci == LN_GROUPS[gi][-1]:
                    nc.scalar.activation(
                        out=et_g[gi],
                        in_=et_g[gi],
                        func=mybir.ActivationFunctionType.Ln,
                        bias=1.0,
                        accum_out=ssp_t[:, gi : gi + 1],
                    )
                    ln_done.add(gi)

            # Export PSUM through SBUF (DMA cannot read PSUM). These DVE
            # copies depend only on the last matmuls, so they overlap the
            # final ACT passes.
            red_sb = stats_pool.tile([1, 5 * MM], f32)
            nc.vector.tensor_copy(out=red_sb[:, 0 : 4 * MM], in_=sxm_ps)
            nc.vector.tensor_copy(out=red_sb[:, 4 * MM : 5 * MM], in_=cnt_ps)
            nc.sync.dma_start(out=red_d[:], in_=red_sb)
            nc.sync.dma_start(out=ssp_d[:], in_=ssp_t)
    nc.finalize()
    return nc


def _get_nc():
    global _nc_cache
    if _nc_cache is None:
        _nc_cache = _build_bass()
    return _nc_cache


def _make_in_maps(cancer_logits, prostate_mask, needle_mask):
    bf16 = ml_dtypes.bfloat16
    x = np.asarray(cancer_logits, dtype=np.float32).reshape(B, P, FD)
    p = np.asarray(prostate_mask, dtype=np.float32).reshape(B, P, FD)
    n = np.asarray(needle_mask, dtype=np.float32).reshape(B, P, FD)
    xpn = np.empty((B, P, 3, FD), dtype=bf16)
    xpn[:, :, 0, :] = p.astype(bf16)
    xpn[:, :, 1, :] = n.astype(bf16)
    xpn[:, :, 2, :] = x.astype(bf16)
    return [
        {"xpn": xpn[c * IMGS_PER_CORE : (c + 1) * IMGS_PER_CORE]}
        for c in range(N_CORES)
    ]


def _combine(results, label):
    y = np.asarray(label, dtype=np.float64).reshape(B)
    ln2 = np.log(2.0)
    num = 0.0
    cnt = 0.0
    for c in range(N_CORES):
        red = np.asarray(results[c]["red"], dtype=np.float64).reshape(5 * MM)
        ssp = np.asarray(results[c]["ssp"], dtype=np.float64)
        sxm_i = red[: 4 * MM].reshape(4, MM).sum(axis=1)  # per image
        c_core = red[4 * MM :].sum()
        a_sum = ssp.sum() - (IMGS_PER_CORE * N_PER_IMG - c_core) * ln2
        y_i = y[c * IMGS_PER_CORE : (c + 1) * IMGS_PER_CORE]
        num += a_sum - (y_i * sxm_i).sum()
        cnt += c_core
    return np.float32(num / max(cnt, 1.0))


def kernel(cancer_logits, label, prostate_mask, needle_mask):
    nc = _get_nc()
    in_maps = _make_in_maps(cancer_logits, prostate_mask, needle_mask)
    res = run_bass_kernel_spmd(nc, in_maps, core_ids=list(range(N_CORES)))
    return _combine(res.results, label)
